# revision 15
# baseline (speedup 1.0000x reference)
"""Trainium2 Bass kernel for DirectionalFreqEmbed (per-token gather + grouped GEMM).

Strategy: token-parallel across 8 NeuronCores (30 tokens/core). Each token's
ragged gather is executed on-device with `dma_gather` (int16 row indices into a
batch-innermost x slab), producing the [l%128, l//128, batch] layout that feeds
the 128-contraction matmul chunks directly. The per-token bias is folded into
the GEMM via a constant ones-row in the gather source and a bias row appended
to the weight tile. W (535 MB total) is read exactly once across the chip.

kernel(**inputs) takes FULL unsharded inputs and returns the FULL output.
"""
import os
import sys

import numpy as np

for _p in ("/opt/trn_rl_repo", "/root/.axon_site/_ro/trn_rl_repo"):
    if os.path.isdir(_p) and _p not in sys.path:
        sys.path.insert(0, _p)

try:  # the staged antenv lacks axon_hooks; inject a functional stand-in so
    import antenv.axon_hooks  # noqa: F401  (trace=True degrades instead of crashing)
except ImportError:
    import types as _types

    _hooks = _types.ModuleType("antenv.axon_hooks")
    _hooks._hook = None
    _hooks.get_axon_ntff_profile_hook = lambda: _hooks._hook
    _hooks.set_axon_ntff_profile_hook = lambda h: setattr(_hooks, "_hook", h)
    sys.modules["antenv.axon_hooks"] = _hooks

import concourse.bass as bass
import concourse.tile as tile
from concourse import bacc, mybir
from concourse.bass_utils import run_bass_kernel_spmd
from concourse.tile_rust import add_dep_helper

IMG, CIN, DIM, B = 64, 30, 384, 64
T, Lmax = 240, 1452
NI = 1536                   # padded index count = NCHUNK * 128
NCHUNK = NI // 128          # 12
IDXCOLS = NI // 16          # 96 int16 columns per token
TPC = T // 8                # 30 tokens per core
SLABROWS = 3 * IMG * IMG    # 12288 rows per 3-channel slab (batch-innermost)
ZROW = 2 * SLABROWS         # zeros row
OROW = ZROW + 1             # ones row (bias contraction)
NROWS = OROW + 1            # 24578 rows in x_core

_prog_cache = {}


def _build_program():
    if "nc" in _prog_cache:
        return _prog_cache["nc"]
    from contextlib import ExitStack

    f32 = mybir.dt.float32
    f32r = mybir.dt.float32r
    i16 = mybir.dt.int16

    nc = bacc.Bacc("TRN2", target_bir_lowering=False, debug=False, num_devices=8)
    x_core = nc.dram_tensor("x_core", [NROWS, B], f32, kind="ExternalInput").ap()
    # dma_gather's SWDGE ucode needs a compile-time-fixed source address;
    # bounce the dynamically-bound input into an Internal DRAM scratch.
    x_int = nc.dram_tensor("x_int", [NROWS, B], f32).ap()
    w_core = nc.dram_tensor(
        "w_core", [TPC, 128, NCHUNK * DIM], f32, kind="ExternalInput").ap()
    idx_core = nc.dram_tensor(
        "idx_core", [128, TPC * IDXCOLS], i16, kind="ExternalInput").ap()
    y_core = nc.dram_tensor("y_core", [TPC, B, DIM], f32, kind="ExternalOutput").ap()

    with tile.TileContext(nc) as tc, ExitStack() as ctx:
        idx_pool = ctx.enter_context(tc.tile_pool(name="idx", bufs=1))
        w_pool = ctx.enter_context(tc.tile_pool(name="w", bufs=4))
        g_pool = ctx.enter_context(tc.tile_pool(name="g", bufs=3))
        psum_pool = ctx.enter_context(tc.tile_pool(name="ps", bufs=4, space="PSUM"))
        out_pool = ctx.enter_context(tc.tile_pool(name="o", bufs=3))

        idx_tile = idx_pool.tile([128, TPC * IDXCOLS], i16)
        nc.sync.dma_start(idx_tile[:], idx_core[:])
        xdma = nc.sync.dma_start(x_int[:], x_core[:])

        for j in range(TPC):
            w_tile = w_pool.tile([128, NCHUNK * DIM], f32)
            nc.sync.dma_start(w_tile[:], w_core[j])

            g_tile = g_pool.tile([128, NCHUNK * B], f32)
            if os.environ.get("KERNEL_VARIANT") == "nogather":
                nc.gpsimd.memset(g_tile[:], 0.0)
            else:
                gi = nc.gpsimd.dma_gather(
                    out_ap=g_tile[:].rearrange("p (c e) -> p c e", e=B),
                    in_ap=x_int[:],
                    idxs_ap=idx_tile[:, j * IDXCOLS:(j + 1) * IDXCOLS],
                    num_idxs=NI,
                    num_idxs_reg=NI,
                    elem_size=B,
                    single_packet=False,  # 97 descs > 64-desc packet limit
                )
                add_dep_helper(gi.ins, xdma.ins, reason="gather reads x_int")

            psum_t = psum_pool.tile([B, DIM], f32)
            for ck in range(NCHUNK):
                nc.tensor.matmul(
                    psum_t[:],
                    lhsT=g_tile[:, ck * B:(ck + 1) * B],
                    rhs=w_tile[:, ck * DIM:(ck + 1) * DIM],
                    start=(ck == 0),
                    stop=(ck == NCHUNK - 1),
                )

            o_tile = out_pool.tile([B, DIM], f32)
            nc.vector.tensor_copy(o_tile[:], psum_t[:])
            nc.sync.dma_start(y_core[j], o_tile[:])

    nc.compile()
    _prog_cache["nc"] = nc
    return nc


def _core_token_ids(cgroup):
    """8 lists of 30 token ids, grouped so each core touches only 2 channel
    slabs. cgroup[t] in [0,10) is token t's channel group."""
    tok_by_c = [[] for _ in range(10)]
    for t in range(T):
        tok_by_c[cgroup[t]].append(t)
    assert all(len(v) == 24 for v in tok_by_c)
    cores = []
    for k in range(8):
        r = 8 if k < 4 else 9
        jj = k if k < 4 else k - 4
        cores.append(tok_by_c[k] + tok_by_c[r][jj * 6:(jj + 1) * 6])
    return cores


def _shard(x, W, bias, idx_a, idx_b, idx_c, lens):
    cgroup = (idx_c[:, 0] % 10).astype(np.int64)
    cores = _core_token_ids(cgroup)
    in_maps = []
    for k in range(8):
        r = 8 if k < 4 else 9
        chans = [k, k + 10, k + 20, r, r + 10, r + 20]
        x_core = np.empty((NROWS, B), np.float32)
        x_core[:2 * SLABROWS] = (
            x[:, chans, :, :].transpose(1, 2, 3, 0).reshape(2 * SLABROWS, B))
        x_core[ZROW] = 0.0
        x_core[OROW] = 1.0

        toks = cores[k]
        w_pad = np.zeros((TPC, NI, DIM), np.float32)
        rows = np.full((TPC, NI), ZROW, np.int16)
        for j, t in enumerate(toks):
            L = int(lens[t])
            w_pad[j, :Lmax] = W[t]
            w_pad[j, Lmax] = bias[t]
            slab = 0 if cgroup[t] == k else 1
            rr = (slab * SLABROWS + (idx_c[t, :L] // 10) * (IMG * IMG)
                  + idx_a[t, :L] * IMG + idx_b[t, :L])
            rows[j, :L] = rr.astype(np.int16)
            rows[j, Lmax] = OROW
        w_sh = np.ascontiguousarray(
            w_pad.reshape(TPC, NCHUNK, 128, DIM).transpose(0, 2, 1, 3)
        ).reshape(TPC, 128, NCHUNK * DIM)
        idx16 = rows.reshape(TPC, IDXCOLS, 16).transpose(2, 0, 1).reshape(16, -1)
        # replicated across the 8 gpsimd cores' 16-partition groups
        idx_sh = np.tile(idx16, (8, 1))
        in_maps.append({"x_core": x_core, "w_core": w_sh, "idx_core": idx_sh})
    return in_maps, cores


LAST_RESULTS = None


def kernel(x, W, bias, idx_a, idx_b, idx_c, lens):
    global LAST_RESULTS
    x = np.asarray(x, np.float32)
    W = np.asarray(W, np.float32)
    bias = np.asarray(bias, np.float32)
    idx_a = np.asarray(idx_a, np.int32)
    idx_b = np.asarray(idx_b, np.int32)
    idx_c = np.asarray(idx_c, np.int32)
    lens = np.asarray(lens, np.int32)
    assert x.shape == (B, CIN, IMG, IMG) and W.shape == (T, Lmax, DIM)

    in_maps, cores = _shard(x, W, bias, idx_a, idx_b, idx_c, lens)
    nc = _build_program()
    res = run_bass_kernel_spmd(
        nc, in_maps, core_ids=list(range(8)),
        tmpdir=os.environ.get("KERNEL_TRACE_TMPDIR") or None)
    LAST_RESULTS = res

    y = np.empty((B, T, DIM), np.float32)
    for k in range(8):
        yk = res.results[k]["y_core"]          # [TPC, B, DIM]
        y[:, cores[k], :] = yk.transpose(1, 0, 2)
    return y
